# revision 1
# baseline (speedup 1.0000x reference)
import jax
import jax.numpy as jnp
import numpy as np
from functools import partial

# nn_GCN_17008070492360: N=100000, K=16, F=128, H1=64, H2=32, C=10
# Data-parallel over nodes across 8 NeuronCores; W1/W2/Wc replicated.
N, K, F = 100000, 16, 128
EPS = 1e-12
NCORES = 8


def _l2norm(a):
    n = jnp.linalg.norm(a, axis=-1, keepdims=True)
    return a / jnp.maximum(n, EPS)


@partial(jax.pmap, axis_name="i", in_axes=(0, 0, None, None, None))
def _shard_fn(x, neighbor, W1, W2, Wc):
    x1 = _l2norm(x) @ W1.T                      # [n, H1]
    nb1 = _l2norm(neighbor) @ W1.T              # [n, K, H1]
    agg = jax.nn.relu(nb1.sum(axis=1))          # self path (unused downstream)
    nbs = jax.nn.relu(x1[:, None, :] + nb1)     # [n, K, H1]
    x2 = nbs.sum(axis=1) @ W2.T                 # sum_k (relu(...) @ W2.T)
    del agg
    return jax.nn.relu(x2) @ Wc.T               # [n, C]


def kernel(x, neighbor, W1, W2, Wc):
    x = np.asarray(x, dtype=np.float32)
    neighbor = np.asarray(neighbor, dtype=np.float32)
    n_per = N // NCORES
    xs = x.reshape(NCORES, n_per, F)
    nbs = neighbor.reshape(NCORES, n_per, K, F)
    out = _shard_fn(xs, nbs, jnp.asarray(W1), jnp.asarray(W2), jnp.asarray(Wc))
    return np.asarray(out).reshape(N, -1).astype(np.float32)



# revision 10
# speedup vs baseline: 1.1121x; 1.1121x over previous
"""nn_GCN_17008070492360 — Bass/Tile kernel for Trainium2, 8 NeuronCores.

GCN message passing: N=100000 nodes, K=16 neighbors, F=128 features,
H1=64, H2=32, C=10.  Data-parallel over nodes; tiny weights replicated.

Per-core program (SPMD, 12544 nodes/core with small overlap between cores):
loop over 98 "supertiles" of 128 nodes each.  Within a supertile there are
17 row-chunks of 128 rows x 128 features (chunk 0 = the node features x,
chunks 1..16 = the 16*128 neighbor rows):
  DMA    : load raw fp32 rows (rows -> partitions, features -> free dim)
  GPSIMD : cast fp32 -> bf16
  ACT    : squares (for row norms)
  DVE    : sum-of-squares reduce -> ACT sqrt -> DVE reciprocal = 1/||row||
  DVE    : per-chunk tensor_scalar row-scale (normalize, per-partition scalar)
  PE     : transpose each scaled chunk (features -> partitions) into PSUM
  DVE    : PSUM -> SBUF copy (bf16)
  PE     : W1 matmuls; even/odd chunks packed into PSUM partitions 0:64/64:128
  ACT    : x1 staging copies (shifted halves for the broadcast add)
  DVE    : add x1 (broadcast over k), relu, sum over k (segmented reduce)
  PE     : W2 (row-tiled pair), relu via ACT, Wc (row-tiled pair)
  PE     : transpose back to rows-on-partitions; DMA out
"""

import os
import sys
from contextlib import ExitStack

import numpy as np

try:
    import ml_dtypes
except ImportError:  # pragma: no cover
    ml_dtypes = None

# --- problem constants (hardcoded; kernel.py must be self-contained) -------
N, K, F = 100000, 16, 128
H1, H2, C = 64, 32, 10
NCORES = 8
NODES_PER_CORE = 12544            # 98 supertiles * 128 nodes
NT = 98                           # supertiles per core
CH = 17                           # chunks per supertile (1 x-chunk + 16 nb)
# Core start offsets; consecutive cores overlap slightly (recomputed rows
# produce identical outputs, so later writes are idempotent).
CORE_STARTS = [0, 12492, 24984, 37476, 49968, 62460, 74952, 87456]

_BF16 = None
_NC_CACHE = {}


def _bf16():
    global _BF16
    if _BF16 is None:
        import ml_dtypes as md
        _BF16 = md.bfloat16
    return _BF16


def _build_module(nodes=NODES_PER_CORE):
    """Build the Bass module (one SPMD program, run on all 8 cores)."""
    import concourse.bacc as bacc
    import concourse.bass as bass
    import concourse.tile as tile
    from concourse import mybir

    nt = nodes // 128
    assert nodes % 128 == 0

    f32 = mybir.dt.float32
    bf16 = mybir.dt.bfloat16
    AX = mybir.AxisListType
    ALU = mybir.AluOpType
    ACTF = mybir.ActivationFunctionType

    nc = bacc.Bacc("TRN2", target_bir_lowering=False, debug=False,
                   num_devices=NCORES)

    x_d = nc.declare_dram_parameter("x", [nodes, F], f32,
                                    isOutput=False).ap()
    nb_d = nc.declare_dram_parameter("nb", [nodes * K, F], f32,
                                     isOutput=False).ap()
    w1t_d = nc.declare_dram_parameter("w1t", [F, H1], bf16, isOutput=False).ap()
    w2t_d = nc.declare_dram_parameter("w2t", [2 * H1, H2], bf16,
                                      isOutput=False).ap()
    wct_d = nc.declare_dram_parameter("wct", [2 * H2, C], bf16,
                                      isOutput=False).ap()
    idb_d = nc.declare_dram_parameter("idb", [128, 128], bf16,
                                      isOutput=False).ap()
    idf_d = nc.declare_dram_parameter("idf", [16, 16], f32, isOutput=False).ap()
    out_d = nc.declare_dram_parameter("out", [nodes, C], f32,
                                      isOutput=True).ap()

    with tile.TileContext(nc) as tc:
        with ExitStack() as ctx, \
                nc.allow_low_precision("bf16 intermediates are within the "
                                       "2e-2 tolerance of this problem"):
            _emit_kernel(ctx, tc, nc, mybir, AX, ALU, ACTF, f32, bf16,
                         x_d, nb_d, w1t_d, w2t_d, wct_d, idb_d, idf_d, out_d,
                         nt)
    nc.compile()
    return nc


def _emit_kernel(ctx, tc, nc, mybir, AX, ALU, ACTF, f32, bf16,
                 x_d, nb_d, w1t_d, w2t_d, wct_d, idb_d, idf_d, out_d, nt=NT):
    # ---- constant tiles (loaded once) ------------------------------------
    const = ctx.enter_context(tc.tile_pool(name="const", bufs=1))
    w1t = const.tile([F, H1], bf16, tag="w1t")
    w2t = const.tile([2 * H1, H2], bf16, tag="w2t")
    wct = const.tile([2 * H2, C], bf16, tag="wct")
    idb = const.tile([128, 128], bf16, tag="idb")
    idf = const.tile([16, 16], f32, tag="idf")
    nc.sync.dma_start(w1t[:], w1t_d)
    nc.sync.dma_start(w2t[:], w2t_d)
    nc.sync.dma_start(wct[:], wct_d)
    nc.sync.dma_start(idb[:], idb_d)
    nc.sync.dma_start(idf[:], idf_d)

    # ---- working pools ---------------------------------------------------
    raw_p = ctx.enter_context(tc.tile_pool(name="raw", bufs=2))
    bf_p = ctx.enter_context(tc.tile_pool(name="bfc", bufs=2))
    bfs_p = ctx.enter_context(tc.tile_pool(name="bfs", bufs=2))
    sq_p = ctx.enter_context(tc.tile_pool(name="sq", bufs=2))
    xt_p = ctx.enter_context(tc.tile_pool(name="xt", bufs=2))
    z_p = ctx.enter_context(tc.tile_pool(name="z", bufs=2))
    sm_p = ctx.enter_context(tc.tile_pool(name="sm", bufs=3))
    o_p = ctx.enter_context(tc.tile_pool(name="o", bufs=2))

    psT_p = ctx.enter_context(tc.tile_pool(name="psT", bufs=1, space="PSUM"))
    ps2_p = ctx.enter_context(tc.tile_pool(name="ps2", bufs=1, space="PSUM"))
    psm_p = ctx.enter_context(tc.tile_pool(name="psm", bufs=2, space="PSUM"))

    for t in range(nt):
        r0 = t * 128
        # ---- load ---------------------------------------------------------
        raw = raw_p.tile([128, CH, F], f32, tag="raw")
        nc.sync.dma_start(raw[:, 0, :], x_d[r0:r0 + 128, :])
        nbs = nb_d[t * 2048:(t + 1) * 2048, :]
        nc.sync.dma_start(raw[:, 1:, :],
                          nbs.rearrange("(c p) f -> p c f", p=128))

        # ---- cast to bf16 (GPSIMD) ---------------------------------------
        bfc = bf_p.tile([128, CH, F], bf16, tag="bfc")
        nc.gpsimd.tensor_copy(bfc[:], raw[:])

        # ---- row norms ----------------------------------------------------
        sq = sq_p.tile([128, CH, F], bf16, tag="sq")
        nc.scalar.square(sq[:], raw[:])
        ssq = sm_p.tile([128, CH], f32, tag="ssq")
        nc.vector.tensor_reduce(ssq[:], sq[:], axis=AX.X, op=ALU.add)
        nrm = sm_p.tile([128, CH], f32, tag="nrm")
        nc.scalar.sqrt(nrm[:], ssq[:])
        rno = sm_p.tile([128, CH], f32, tag="rno")
        nc.vector.reciprocal(rno[:], nrm[:])

        # ---- normalize (per-chunk per-partition scale) --------------------
        bfs = bfs_p.tile([128, CH, F], bf16, tag="bfs")
        for c in range(CH):
            nc.vector.tensor_scalar_mul(bfs[:, c, :], bfc[:, c, :],
                                        rno[:, c:c + 1])

        # ---- transpose chunks (PE) ---------------------------------------
        psT = psT_p.tile([128, CH, 128], bf16, tag="psT")
        for c in range(CH):
            nc.tensor.transpose(psT[:, c, :], bfs[:, c, :], idb[:])

        # ---- PSUM -> SBUF (DVE) ------------------------------------------
        xt = xt_p.tile([128, CH, 128], bf16, tag="xt")
        nc.vector.tensor_copy(xt[:], psT[:])

        # ---- W1 matmuls ---------------------------------------------------
        # psum2 cols 0:1024 = nb pairs (even chunks in partitions 0:64,
        # odd chunks in partitions 64:128); cols 1024:1152 = x1 (0:64).
        ps2 = ps2_p.tile([128, 1152], f32, tag="ps2")
        for g in range(2):            # pair groups (4 pairs each)
            for h in range(2):        # even/odd half
                rhs = xt[:, 1 + 8 * g + h:8 + 8 * g + h:2, :]
                nc.tensor.matmul(ps2[64 * h:64 * (h + 1),
                                     512 * g:512 * (g + 1)],
                                 w1t[:], rhs,
                                 start=True, stop=True)
        nc.tensor.matmul(ps2[0:64, 1024:1152], w1t[:], xt[:, 0, :],
                         start=True, stop=True)

        # ---- x1 staging (shifted halves) ---------------------------------
        z0x2 = sm_p.tile([128, 128], bf16, tag="z0x2")
        nc.scalar.copy(z0x2[0:64, 0:128], ps2[0:64, 1024:1152])
        nc.scalar.copy(z0x2[64:128, 0:120], ps2[0:64, 1032:1152])

        # ---- z = relu(nb1 + x1), then sum over k -------------------------
        z1 = z_p.tile([128, 1024], bf16, tag="z1")
        in0 = ps2[:, 0:1024].rearrange("p (pr j k) -> p pr j k", pr=8, j=8)
        in1 = z0x2[:].rearrange("p (pr j) -> p pr j", pr=8)[:, :, 0:8]
        in1 = in1.unsqueeze(3).broadcast_to([128, 8, 8, K])
        zv = z1[:].rearrange("p (pr j k) -> p pr j k", pr=8, j=8)
        nc.vector.tensor_tensor(zv, in0, in1, op=ALU.add)
        z2 = z_p.tile([128, 1024], bf16, tag="z2")
        nc.vector.tensor_scalar_max(z2[:], z1[:], 0.0)
        s2 = sm_p.tile([128, 64], bf16, tag="s2")
        nc.vector.tensor_reduce(s2[:], z2[:].rearrange("p (a k) -> p a k",
                                                       k=K),
                                axis=AX.X, op=ALU.add)

        # ---- W2 + relu ----------------------------------------------------
        ps3 = psm_p.tile([128, 512], f32, tag="psm")
        nc.tensor.matmul(ps3[0:32, 0:64], w2t[0:64, :], s2[0:64, :],
                         start=True, stop=True, tile_position=(0, 0))
        nc.tensor.matmul(ps3[32:64, 0:64], w2t[64:128, :], s2[64:128, :],
                         start=True, stop=True, tile_position=(64, 32))
        h2r = sm_p.tile([64, 64], bf16, tag="h2r")
        nc.scalar.activation(h2r[:], ps3[0:64, 0:64], ACTF.Relu)

        # ---- Wc -----------------------------------------------------------
        ps4 = psm_p.tile([128, 512], f32, tag="psm")
        nc.tensor.matmul(ps4[0:C, 0:64], wct[0:H2, :], h2r[0:32, :],
                         start=True, stop=True, tile_position=(0, 0))
        nc.tensor.matmul(ps4[32:32 + C, 0:64], wct[H2:2 * H2, :],
                         h2r[32:64, :],
                         start=True, stop=True, tile_position=(32, 32))

        # ---- gather outputs into [C, 128] then transpose back ------------
        o1 = o_p.tile([C, 128], f32, tag="o1")
        oe = o1[:].rearrange("p (pr j) -> p pr j", pr=8)
        nc.vector.tensor_copy(oe[:, :, 0:8],
                              ps4[0:C, 0:64].rearrange("p (a b) -> p a b",
                                                       a=8))
        nc.vector.tensor_copy(oe[:, :, 8:16],
                              ps4[32:32 + C, 0:64].rearrange("p (a b) -> p a b",
                                                             a=8))
        ps5 = psm_p.tile([128, 512], f32, tag="psm")
        nc.tensor.transpose(ps5[0:128, 0:C], o1[:], idf[0:C, 0:C])
        obuf = o_p.tile([128, C], f32, tag="obuf")
        nc.vector.tensor_copy(obuf[:], ps5[0:128, 0:C])
        nc.sync.dma_start(out_d[r0:r0 + 128, :], obuf[:])


def _get_nc():
    if "nc" not in _NC_CACHE:
        _NC_CACHE["nc"] = _build_module()
    return _NC_CACHE["nc"]


def _prep_weights(W1, W2, Wc):
    bf = _bf16()
    w1t = np.ascontiguousarray(W1.T).astype(bf)                 # [128, 64]
    w2t = np.concatenate([W2.T, W2.T], axis=0).astype(bf)       # [128, 32]
    wct = np.concatenate([Wc.T, Wc.T], axis=0).astype(bf)       # [64, 10]
    idb = np.eye(128, dtype=np.float32).astype(bf)
    idf = np.eye(16, dtype=np.float32)
    return w1t, w2t, wct, idb, idf


def kernel(x, neighbor, W1, W2, Wc):
    from concourse.bass_utils import run_bass_kernel_spmd

    x = np.asarray(x, np.float32)
    neighbor = np.asarray(neighbor, np.float32)
    w1t, w2t, wct, idb, idf = _prep_weights(np.asarray(W1, np.float32),
                                            np.asarray(W2, np.float32),
                                            np.asarray(Wc, np.float32))
    nc = _get_nc()
    in_maps = []
    for s in CORE_STARTS:
        in_maps.append({
            "x": x[s:s + NODES_PER_CORE],
            "nb": neighbor[s:s + NODES_PER_CORE].reshape(-1, F),
            "w1t": w1t, "w2t": w2t, "wct": wct, "idb": idb, "idf": idf,
        })
    res = run_bass_kernel_spmd(nc, in_maps, list(range(NCORES)))
    out = np.empty((N, C), np.float32)
    for s, r in zip(CORE_STARTS, res.results):
        out[s:s + NODES_PER_CORE] = r["out"]
    return out


# revision 22
# speedup vs baseline: 23725.8841x; 21333.4457x over previous
"""nn_GCN_17008070492360 — Bass/Tile kernel for Trainium2, 8 NeuronCores.

GCN message passing: N=100000 nodes, K=16 neighbors, F=128 features,
H1=64, H2=32, C=10.  Data-parallel over nodes; tiny weights replicated.

Per-core program (SPMD, 12544 nodes/core with small overlap between cores):
loop over 98 "supertiles" of 128 nodes each.  Within a supertile there are
17 row-chunks of 128 rows x 128 features (chunk 0 = the node features x,
chunks 1..16 = the 16*128 neighbor rows):
  DMA    : load raw fp32 rows (rows -> partitions, features -> free dim)
  ACT    : squares (bf16, for row norms)
  DVE    : sum-of-squares reduce -> ACT sqrt -> DVE reciprocal = 1/||row||
  DVE    : one batched tensor_tensor multiply = normalize + fp32->bf16 cast
  PE     : transpose each scaled chunk (features -> partitions) into PSUM
  ACT    : PSUM -> SBUF copy of the transposed chunks (bf16)
  PE     : W1 matmuls, even/odd chunks packed to PSUM partitions 0:64/64:128;
           the x1 broadcast-add is folded in as accumulating matmuls whose
           rhs repeats each node's column K times (stride-0 AP dim)
  ACT    : relu evacuates W1 PSUM -> SBUF bf16
  DVE    : segmented reduce over k (sum of the 16 relu'd neighbor terms)
  PE     : W2 (row-tiled pair), relu via ACT, Wc (row-tiled pair)
  PE     : transpose back to rows-on-partitions; DMA out
Measured on trn2 hardware: ~0.80 ms per core for the full 100k-node pass
(DVE-bound; DMA floor for the 109 MB/core of HBM reads is ~0.3-0.5 ms).
"""

import os
import sys
from contextlib import ExitStack

import numpy as np

try:
    import ml_dtypes
except ImportError:  # pragma: no cover
    ml_dtypes = None

# --- problem constants (hardcoded; kernel.py must be self-contained) -------
N, K, F = 100000, 16, 128
H1, H2, C = 64, 32, 10
NCORES = 8
NODES_PER_CORE = 12544            # 98 supertiles * 128 nodes
NT = 98                           # supertiles per core
CH = 17                           # chunks per supertile (1 x-chunk + 16 nb)
# Core start offsets; consecutive cores overlap slightly (recomputed rows
# produce identical outputs, so later writes are idempotent).
CORE_STARTS = [0, 12492, 24984, 37476, 49968, 62460, 74952, 87456]

_BF16 = None
_NC_CACHE = {}


def _bf16():
    global _BF16
    if _BF16 is None:
        import ml_dtypes as md
        _BF16 = md.bfloat16
    return _BF16


def _build_module(nodes=NODES_PER_CORE):
    """Build the Bass module (one SPMD program, run on all 8 cores)."""
    import concourse.bacc as bacc
    import concourse.bass as bass
    import concourse.tile as tile
    from concourse import mybir

    nt = nodes // 128
    assert nodes % 128 == 0

    f32 = mybir.dt.float32
    bf16 = mybir.dt.bfloat16
    AX = mybir.AxisListType
    ALU = mybir.AluOpType
    ACTF = mybir.ActivationFunctionType

    nc = bacc.Bacc("TRN2", target_bir_lowering=False, debug=False,
                   num_devices=NCORES)

    x_d = nc.declare_dram_parameter("x", [nodes, F], f32,
                                    isOutput=False).ap()
    nb_d = nc.declare_dram_parameter("nb", [nodes * K, F], f32,
                                     isOutput=False).ap()
    w1t_d = nc.declare_dram_parameter("w1t", [F, H1], bf16, isOutput=False).ap()
    w2t_d = nc.declare_dram_parameter("w2t", [2 * H1, H2], bf16,
                                      isOutput=False).ap()
    wct_d = nc.declare_dram_parameter("wct", [2 * H2, C], bf16,
                                      isOutput=False).ap()
    idb_d = nc.declare_dram_parameter("idb", [128, 128], bf16,
                                      isOutput=False).ap()
    idf_d = nc.declare_dram_parameter("idf", [16, 16], f32, isOutput=False).ap()
    out_d = nc.declare_dram_parameter("out", [nodes, C], f32,
                                      isOutput=True).ap()

    with tile.TileContext(nc) as tc:
        with ExitStack() as ctx, \
                nc.allow_low_precision("bf16 intermediates are within the "
                                       "2e-2 tolerance of this problem"):
            _emit_kernel(ctx, tc, nc, mybir, AX, ALU, ACTF, f32, bf16,
                         x_d, nb_d, w1t_d, w2t_d, wct_d, idb_d, idf_d, out_d,
                         nt)
    nc.compile()
    return nc


def _emit_kernel(ctx, tc, nc, mybir, AX, ALU, ACTF, f32, bf16,
                 x_d, nb_d, w1t_d, w2t_d, wct_d, idb_d, idf_d, out_d, nt=NT):
    # ---- constant tiles (loaded once) ------------------------------------
    const = ctx.enter_context(tc.tile_pool(name="const", bufs=1))
    w1t = const.tile([F, H1], bf16, tag="w1t")
    w2t = const.tile([2 * H1, H2], bf16, tag="w2t")
    wct = const.tile([2 * H2, C], bf16, tag="wct")
    idb = const.tile([128, 128], bf16, tag="idb")
    idf = const.tile([16, 16], f32, tag="idf")
    nc.sync.dma_start(w1t[:], w1t_d)
    nc.sync.dma_start(w2t[:], w2t_d)
    nc.sync.dma_start(wct[:], wct_d)
    nc.sync.dma_start(idb[:], idb_d)
    nc.sync.dma_start(idf[:], idf_d)

    # ---- working pools ---------------------------------------------------
    raw_p = ctx.enter_context(tc.tile_pool(name="raw", bufs=2))
    bfs_p = ctx.enter_context(tc.tile_pool(name="bfs", bufs=2))
    sq_p = ctx.enter_context(tc.tile_pool(name="sq", bufs=2))
    xt_p = ctx.enter_context(tc.tile_pool(name="xt", bufs=2))
    z_p = ctx.enter_context(tc.tile_pool(name="z", bufs=2))
    sm_p = ctx.enter_context(tc.tile_pool(name="sm", bufs=3))
    o_p = ctx.enter_context(tc.tile_pool(name="o", bufs=2))

    psT_p = ctx.enter_context(tc.tile_pool(name="psT", bufs=1, space="PSUM"))
    ps2_p = ctx.enter_context(tc.tile_pool(name="ps2", bufs=2, space="PSUM"))
    psm_p = ctx.enter_context(tc.tile_pool(name="psm", bufs=1, space="PSUM"))

    for t in range(nt):
        r0 = t * 128
        # ---- load ---------------------------------------------------------
        raw = raw_p.tile([128, CH, F], f32, tag="raw")
        nc.sync.dma_start(raw[:, 0, :], x_d[r0:r0 + 128, :])
        nbs = nb_d[t * 2048:(t + 1) * 2048, :]
        # Row-pair layout: partition p of pair-group s holds DRAM rows
        # 256s + 2p and 256s + 2p + 1 (chunks 1+2s and 2+2s) -> each DMA
        # descriptor moves 1 KiB contiguous instead of 512 B.
        nc.sync.dma_start(
            raw[:, 1:, :].rearrange("p (s rp) f -> p s rp f", rp=2),
            nbs.rearrange("(s p rp) f -> p s rp f", p=128, rp=2))

        # ---- row norms ----------------------------------------------------
        sq = sq_p.tile([128, CH, F], bf16, tag="sq")
        nc.scalar.square(sq[:], raw[:])
        ssq = sm_p.tile([128, CH], f32, tag="ssq")
        nc.vector.tensor_reduce(ssq[:], sq[:], axis=AX.X, op=ALU.add)
        nrm = sm_p.tile([128, CH], f32, tag="nrm")
        nc.scalar.sqrt(nrm[:], ssq[:])
        rno = sm_p.tile([128, CH], f32, tag="rno")
        nc.vector.reciprocal(rno[:], nrm[:])

        # ---- normalize + cast in one batched TT (scale bcast over F) -----
        bfs = bfs_p.tile([128, CH, F], bf16, tag="bfs")
        rnob = rno[:].unsqueeze(2).broadcast_to([128, CH, F])
        nc.vector.tensor_tensor(bfs[:], raw[:], rnob, op=ALU.mult)

        # ---- transpose chunks (PE) ---------------------------------------
        psT = psT_p.tile([128, CH, 128], bf16, tag="psT")
        for c in range(CH):
            nc.tensor.transpose(psT[:, c, :], bfs[:, c, :], idb[:])

        # ---- PSUM -> SBUF (ACT; DVE is the busier engine) ----------------
        xt = xt_p.tile([128, CH, 128], bf16, tag="xt")
        nc.scalar.copy(xt[:], psT[:])

        # ---- W1 matmuls ---------------------------------------------------
        # psum2 cols 0:1024 = nb pairs (even chunks in partitions 0:64,
        # odd chunks in partitions 64:128).  The x1 term is accumulated
        # directly into PSUM by a second matmul whose rhs repeats each
        # node's column K times (stride-0 AP dim).
        ps2 = ps2_p.tile([128, 1024], f32, tag="ps2")
        xt0 = xt[:, 0, :].rearrange("p (s nj) -> p s nj", s=8)
        for g in range(2):            # pair-group halves (4 groups each)
            for h in range(2):        # rp (row parity) half
                rhs = xt[:, 1 + 8 * g + h:8 + 8 * g + h:2, :]
                nc.tensor.matmul(ps2[64 * h:64 * (h + 1),
                                     512 * g:512 * (g + 1)],
                                 w1t[:], rhs,
                                 start=True, stop=False)
                xrhs = xt0[:, 4 * g:4 * g + 4, :]
                xrhs = xrhs.unsqueeze(3).broadcast_to([128, 4, 16, 8])
                nc.tensor.matmul(ps2[64 * h:64 * (h + 1),
                                     512 * g:512 * (g + 1)],
                                 w1t[:], xrhs,
                                 start=False, stop=True)

        # ---- z = relu(nb1 + x1) evacuated by ACT, then sum over k --------
        # col = 128 s + 8 nj + kk; k = 2 kk + rp(half).  The two partition
        # halves hold complementary k subsets, summed in the W2 matmuls.
        z2 = z_p.tile([128, 1024], bf16, tag="z2")
        nc.scalar.activation(z2[:], ps2[:, 0:1024], ACTF.Relu)
        s2 = sm_p.tile([128, 128], bf16, tag="s2")
        nc.vector.tensor_reduce(s2[:], z2[:].rearrange("p (a kk) -> p a kk",
                                                       kk=8),
                                axis=AX.X, op=ALU.add)

        # ---- W2 (accumulate both k-subset halves) + relu ------------------
        ps3 = psm_p.tile([128, 512], f32, tag="psm")
        nc.tensor.matmul(ps3[0:32, 0:128], w2t[0:64, :], s2[0:64, :],
                         start=True, stop=False, tile_position=(0, 0))
        nc.tensor.matmul(ps3[0:32, 0:128], w2t[64:128, :], s2[64:128, :],
                         start=False, stop=True, tile_position=(64, 0))
        h2r = sm_p.tile([32, 128], bf16, tag="h2r")
        nc.scalar.activation(h2r[:], ps3[0:32, 0:128], ACTF.Relu)

        # ---- Wc -----------------------------------------------------------
        ps4 = psm_p.tile([128, 512], f32, tag="psm")
        nc.tensor.matmul(ps4[0:C, 0:128], wct[0:H2, :], h2r[:],
                         start=True, stop=True, tile_position=(0, 0))

        # ---- transpose [C, nodes] back to [nodes, C] and store -----------
        o1 = o_p.tile([C, 128], f32, tag="o1")
        nc.vector.tensor_copy(o1[:], ps4[0:C, 0:128])
        ps5 = psm_p.tile([128, 512], f32, tag="psm")
        nc.tensor.transpose(ps5[0:128, 0:C], o1[:], idf[0:C, 0:C])
        obuf = o_p.tile([128, C], f32, tag="obuf")
        nc.vector.tensor_copy(obuf[:], ps5[0:128, 0:C])
        nc.sync.dma_start(out_d[r0:r0 + 128, :], obuf[:])


def _get_nc():
    if "nc" not in _NC_CACHE:
        _NC_CACHE["nc"] = _build_module()
    return _NC_CACHE["nc"]


def _prep_weights(W1, W2, Wc):
    bf = _bf16()
    w1t = np.ascontiguousarray(W1.T).astype(bf)                 # [128, 64]
    w2t = np.concatenate([W2.T, W2.T], axis=0).astype(bf)       # [128, 32]
    wct = np.concatenate([Wc.T, Wc.T], axis=0).astype(bf)       # [64, 10]
    idb = np.eye(128, dtype=np.float32).astype(bf)
    idf = np.eye(16, dtype=np.float32)
    return w1t, w2t, wct, idb, idf


def kernel(x, neighbor, W1, W2, Wc):
    from concourse.bass_utils import run_bass_kernel_spmd

    x = np.asarray(x, np.float32)
    neighbor = np.asarray(neighbor, np.float32)
    w1t, w2t, wct, idb, idf = _prep_weights(np.asarray(W1, np.float32),
                                            np.asarray(W2, np.float32),
                                            np.asarray(Wc, np.float32))
    nc = _get_nc()
    in_maps = []
    for s in CORE_STARTS:
        in_maps.append({
            "x": x[s:s + NODES_PER_CORE],
            "nb": neighbor[s:s + NODES_PER_CORE].reshape(-1, F),
            "w1t": w1t, "w2t": w2t, "wct": wct, "idb": idb, "idf": idf,
        })
    res = run_bass_kernel_spmd(nc, in_maps, list(range(NCORES)))
    out = np.empty((N, C), np.float32)
    for s, r in zip(CORE_STARTS, res.results):
        out[s:s + NODES_PER_CORE] = r["out"]
    return out


# revision 25
# speedup vs baseline: 23835.3917x; 1.0046x over previous
"""nn_GCN_17008070492360 — Bass/Tile kernel for Trainium2, 8 NeuronCores.

GCN message passing: N=100000 nodes, K=16 neighbors, F=128 features,
H1=64, H2=32, C=10.  Data-parallel over nodes; tiny weights replicated.

Per-core program (SPMD, 12544 nodes/core with small overlap between cores):
loop over 98 "supertiles" of 128 nodes each.  Within a supertile there are
17 row-chunks of 128 rows x 128 features (chunk 0 = the node features x,
chunks 1..16 = the 16*128 neighbor rows):
  DMA    : load raw fp32 rows (rows -> partitions, features -> free dim)
  ACT    : squares (bf16, for row norms)
  DVE    : sum-of-squares reduce -> ACT sqrt -> DVE reciprocal = 1/||row||
  DVE    : one batched tensor_tensor multiply = normalize + fp32->bf16 cast
  PE     : transpose each scaled chunk (features -> partitions) into PSUM
  ACT    : PSUM -> SBUF copy of the transposed chunks (bf16)
  PE     : W1 matmuls, even/odd chunks packed to PSUM partitions 0:64/64:128;
           the x1 broadcast-add is folded in as accumulating matmuls whose
           rhs repeats each node's column K times (stride-0 AP dim)
  ACT    : relu evacuates W1 PSUM -> SBUF bf16
  DVE    : segmented reduce over k (sum of the 16 relu'd neighbor terms)
  PE     : W2 (row-tiled pair), relu via ACT, Wc (row-tiled pair)
  PE     : transpose back to rows-on-partitions; DMA out
Measured on trn2 hardware: ~0.80 ms per core for the full 100k-node pass
(DVE-bound; DMA floor for the 109 MB/core of HBM reads is ~0.3-0.5 ms).
"""

import os
import sys
from contextlib import ExitStack

import numpy as np

try:
    import ml_dtypes
except ImportError:  # pragma: no cover
    ml_dtypes = None

# --- problem constants (hardcoded; kernel.py must be self-contained) -------
N, K, F = 100000, 16, 128
H1, H2, C = 64, 32, 10
NCORES = 8
NODES_PER_CORE = 12544            # 98 supertiles * 128 nodes
NT = 98                           # supertiles per core
CH = 17                           # chunks per supertile (1 x-chunk + 16 nb)
# Core start offsets; consecutive cores overlap slightly (recomputed rows
# produce identical outputs, so later writes are idempotent).
CORE_STARTS = [0, 12492, 24984, 37476, 49968, 62460, 74952, 87456]

_BF16 = None
_NC_CACHE = {}


def _bf16():
    global _BF16
    if _BF16 is None:
        import ml_dtypes as md
        _BF16 = md.bfloat16
    return _BF16


def _build_module(nodes=NODES_PER_CORE):
    """Build the Bass module (one SPMD program, run on all 8 cores)."""
    import concourse.bacc as bacc
    import concourse.bass as bass
    import concourse.tile as tile
    from concourse import mybir

    nt = nodes // 128
    assert nodes % 128 == 0

    f32 = mybir.dt.float32
    bf16 = mybir.dt.bfloat16
    AX = mybir.AxisListType
    ALU = mybir.AluOpType
    ACTF = mybir.ActivationFunctionType

    nc = bacc.Bacc("TRN2", target_bir_lowering=False, debug=False,
                   num_devices=NCORES)

    x_d = nc.declare_dram_parameter("x", [nodes, F], f32,
                                    isOutput=False).ap()
    nb_d = nc.declare_dram_parameter("nb", [nodes * K, F], f32,
                                     isOutput=False).ap()
    w1t_d = nc.declare_dram_parameter("w1t", [F, H1], bf16, isOutput=False).ap()
    w2t_d = nc.declare_dram_parameter("w2t", [2 * H1, H2], bf16,
                                      isOutput=False).ap()
    wct_d = nc.declare_dram_parameter("wct", [2 * H2, C], bf16,
                                      isOutput=False).ap()
    idb_d = nc.declare_dram_parameter("idb", [128, 128], bf16,
                                      isOutput=False).ap()
    idf_d = nc.declare_dram_parameter("idf", [16, 16], f32, isOutput=False).ap()
    out_d = nc.declare_dram_parameter("out", [nodes, C], f32,
                                      isOutput=True).ap()

    with tile.TileContext(nc) as tc:
        with ExitStack() as ctx, \
                nc.allow_low_precision("bf16 intermediates are within the "
                                       "2e-2 tolerance of this problem"):
            _emit_kernel(ctx, tc, nc, mybir, AX, ALU, ACTF, f32, bf16,
                         x_d, nb_d, w1t_d, w2t_d, wct_d, idb_d, idf_d, out_d,
                         nt)
    nc.compile()
    return nc


def _emit_kernel(ctx, tc, nc, mybir, AX, ALU, ACTF, f32, bf16,
                 x_d, nb_d, w1t_d, w2t_d, wct_d, idb_d, idf_d, out_d, nt=NT):
    # ---- constant tiles (loaded once) ------------------------------------
    const = ctx.enter_context(tc.tile_pool(name="const", bufs=1))
    w1t = const.tile([F, H1], bf16, tag="w1t")
    w2t = const.tile([2 * H1, H2], bf16, tag="w2t")
    wct = const.tile([2 * H2, C], bf16, tag="wct")
    idb = const.tile([128, 128], bf16, tag="idb")
    idf = const.tile([16, 16], f32, tag="idf")
    nc.sync.dma_start(w1t[:], w1t_d)
    nc.sync.dma_start(w2t[:], w2t_d)
    nc.sync.dma_start(wct[:], wct_d)
    nc.sync.dma_start(idb[:], idb_d)
    nc.sync.dma_start(idf[:], idf_d)

    # ---- working pools ---------------------------------------------------
    raw_p = ctx.enter_context(tc.tile_pool(name="raw", bufs=3))
    bfs_p = ctx.enter_context(tc.tile_pool(name="bfs", bufs=3))
    sq_p = ctx.enter_context(tc.tile_pool(name="sq", bufs=3))
    xt_p = ctx.enter_context(tc.tile_pool(name="xt", bufs=3))
    z_p = ctx.enter_context(tc.tile_pool(name="z", bufs=3))
    sm_p = ctx.enter_context(tc.tile_pool(name="sm", bufs=4))
    o_p = ctx.enter_context(tc.tile_pool(name="o", bufs=2))

    psT_p = ctx.enter_context(tc.tile_pool(name="psT", bufs=1, space="PSUM"))
    ps2_p = ctx.enter_context(tc.tile_pool(name="ps2", bufs=2, space="PSUM"))
    psm_p = ctx.enter_context(tc.tile_pool(name="psm", bufs=1, space="PSUM"))

    for t in range(nt):
        r0 = t * 128
        # ---- load ---------------------------------------------------------
        raw = raw_p.tile([128, CH, F], f32, tag="raw")
        nc.sync.dma_start(raw[:, 0, :], x_d[r0:r0 + 128, :])
        nbs = nb_d[t * 2048:(t + 1) * 2048, :]
        nc.sync.dma_start(raw[:, 1:, :],
                          nbs.rearrange("(c p) f -> p c f", p=128))

        # ---- row norms ----------------------------------------------------
        sq = sq_p.tile([128, CH, F], bf16, tag="sq")
        nc.scalar.square(sq[:], raw[:])
        ssq = sm_p.tile([128, CH], f32, tag="ssq")
        nc.vector.tensor_reduce(ssq[:], sq[:], axis=AX.X, op=ALU.add)
        nrm = sm_p.tile([128, CH], f32, tag="nrm")
        nc.scalar.sqrt(nrm[:], ssq[:])
        rno = sm_p.tile([128, CH], f32, tag="rno")
        nc.vector.reciprocal(rno[:], nrm[:])

        # ---- normalize + cast in one batched TT (scale bcast over F) -----
        bfs = bfs_p.tile([128, CH, F], bf16, tag="bfs")
        rnob = rno[:].unsqueeze(2).broadcast_to([128, CH, F])
        nc.vector.tensor_tensor(bfs[:], raw[:], rnob, op=ALU.mult)

        # ---- transpose chunks (PE) ---------------------------------------
        psT = psT_p.tile([128, CH, 128], bf16, tag="psT")
        for c in range(CH):
            nc.tensor.transpose(psT[:, c, :], bfs[:, c, :], idb[:])

        # ---- PSUM -> SBUF (ACT; DVE is the busier engine) ----------------
        xt = xt_p.tile([128, CH, 128], bf16, tag="xt")
        nc.scalar.copy(xt[:], psT[:])

        # ---- W1 matmuls ---------------------------------------------------
        # psum2 cols 0:1024 = nb pairs (even chunks in partitions 0:64,
        # odd chunks in partitions 64:128).  The x1 term is accumulated
        # directly into PSUM by a second matmul whose rhs repeats each
        # node's column K times (stride-0 AP dim).
        ps2 = ps2_p.tile([128, 1024], f32, tag="ps2")
        xt0 = xt[:, 0, :].rearrange("p (pr jj) -> p pr jj", pr=8)
        for g in range(2):            # pair groups (4 pairs each)
            for h in range(2):        # even/odd half
                rhs = xt[:, 1 + 8 * g + h:8 + 8 * g + h:2, :]
                nc.tensor.matmul(ps2[64 * h:64 * (h + 1),
                                     512 * g:512 * (g + 1)],
                                 w1t[:], rhs,
                                 start=True, stop=False)
                xrhs = xt0[:, 4 * g:4 * g + 4, 8 * h:8 * h + 8]
                xrhs = xrhs.unsqueeze(3).broadcast_to([128, 4, 8, K])
                nc.tensor.matmul(ps2[64 * h:64 * (h + 1),
                                     512 * g:512 * (g + 1)],
                                 w1t[:], xrhs,
                                 start=False, stop=True)

        # ---- z = relu(nb1 + x1) evacuated by ACT, then sum over k --------
        z2 = z_p.tile([128, 1024], bf16, tag="z2")
        nc.scalar.activation(z2[:], ps2[:, 0:1024], ACTF.Relu)
        s2 = sm_p.tile([128, 64], bf16, tag="s2")
        nc.vector.tensor_reduce(s2[:], z2[:].rearrange("p (a k) -> p a k",
                                                       k=K),
                                axis=AX.X, op=ALU.add)

        # ---- W2 + relu ----------------------------------------------------
        ps3 = psm_p.tile([128, 512], f32, tag="psm")
        nc.tensor.matmul(ps3[0:32, 0:64], w2t[0:64, :], s2[0:64, :],
                         start=True, stop=True, tile_position=(0, 0))
        nc.tensor.matmul(ps3[32:64, 0:64], w2t[64:128, :], s2[64:128, :],
                         start=True, stop=True, tile_position=(64, 32))
        h2r = sm_p.tile([64, 64], bf16, tag="h2r")
        nc.scalar.activation(h2r[:], ps3[0:64, 0:64], ACTF.Relu)

        # ---- Wc -----------------------------------------------------------
        ps4 = psm_p.tile([128, 512], f32, tag="psm")
        nc.tensor.matmul(ps4[0:C, 0:64], wct[0:H2, :], h2r[0:32, :],
                         start=True, stop=True, tile_position=(0, 0))
        nc.tensor.matmul(ps4[32:32 + C, 0:64], wct[H2:2 * H2, :],
                         h2r[32:64, :],
                         start=True, stop=True, tile_position=(32, 32))

        # ---- gather outputs into [C, 128] then transpose back ------------
        o1 = o_p.tile([C, 128], f32, tag="o1")
        oe = o1[:].rearrange("p (pr j) -> p pr j", pr=8)
        nc.vector.tensor_copy(oe[:, :, 0:8],
                              ps4[0:C, 0:64].rearrange("p (a b) -> p a b",
                                                       a=8))
        nc.vector.tensor_copy(oe[:, :, 8:16],
                              ps4[32:32 + C, 0:64].rearrange("p (a b) -> p a b",
                                                             a=8))
        ps5 = psm_p.tile([128, 512], f32, tag="psm")
        nc.tensor.transpose(ps5[0:128, 0:C], o1[:], idf[0:C, 0:C])
        obuf = o_p.tile([128, C], f32, tag="obuf")
        nc.vector.tensor_copy(obuf[:], ps5[0:128, 0:C])
        nc.sync.dma_start(out_d[r0:r0 + 128, :], obuf[:])


def _get_nc():
    if "nc" not in _NC_CACHE:
        _NC_CACHE["nc"] = _build_module()
    return _NC_CACHE["nc"]


def _prep_weights(W1, W2, Wc):
    bf = _bf16()
    w1t = np.ascontiguousarray(W1.T).astype(bf)                 # [128, 64]
    w2t = np.concatenate([W2.T, W2.T], axis=0).astype(bf)       # [128, 32]
    wct = np.concatenate([Wc.T, Wc.T], axis=0).astype(bf)       # [64, 10]
    idb = np.eye(128, dtype=np.float32).astype(bf)
    idf = np.eye(16, dtype=np.float32)
    return w1t, w2t, wct, idb, idf


def kernel(x, neighbor, W1, W2, Wc):
    from concourse.bass_utils import run_bass_kernel_spmd

    x = np.asarray(x, np.float32)
    neighbor = np.asarray(neighbor, np.float32)
    w1t, w2t, wct, idb, idf = _prep_weights(np.asarray(W1, np.float32),
                                            np.asarray(W2, np.float32),
                                            np.asarray(Wc, np.float32))
    nc = _get_nc()
    in_maps = []
    for s in CORE_STARTS:
        in_maps.append({
            "x": x[s:s + NODES_PER_CORE],
            "nb": neighbor[s:s + NODES_PER_CORE].reshape(-1, F),
            "w1t": w1t, "w2t": w2t, "wct": wct, "idb": idb, "idf": idf,
        })
    res = run_bass_kernel_spmd(nc, in_maps, list(range(NCORES)))
    out = np.empty((N, C), np.float32)
    for s, r in zip(CORE_STARTS, res.results):
        out[s:s + NODES_PER_CORE] = r["out"]
    return out
